# revision 32
# baseline (speedup 1.0000x reference)
"""AdaFocalLoss on 8 Trainium2 NeuronCores (Bass/Tile, SPMD).

Data-parallel over the batch axis, per the sharding hint: each core gets
8192 of the 65536 logit rows, the 15-entry gamma table is replicated, and
the per-core partial sums are combined on the host (the gather/unshard
step; the reduction over rows is order-independent).

Per-core kernel structure:
  - Rows are assigned to (slot, partition) SORTED BY TARGET on the host:
    slot s holds the 128 rows whose targets sit near the s-th quantile of
    the target distribution.  The row order is free to choose (the final
    loss is a sum over rows), and sorting makes the target-logit gather
    cheap: all 128 targets of a slot fall inside a static 64-column
    window around the slot's quantile center.
  - The shard streams as 64 contiguous 512 KB DMAs (one per slot).  The
    first two ride the Scalar engine's HWDGE ring while the rest queue on
    the Sync ring, so both rings fill the SDMA engines from t=0.  Every
    constant (window-relative iota, window-relative targets, telescoped
    gamma sign/magnitude deltas, bin thresholds) is precomputed on the
    host into one small [P, .] tensor - no device-side constant setup.
  - ScalarE computes exp(x) for every element (fp16 out; the only engine
    with transcendentals).  The per-row sum of exps comes from the
    ACTIVATE's accum_out for a few slots (cheap marginal cost) and from
    VectorE for the rest, as a two-stage reduce: a 2x-mode fp16 add of
    the tile's halves, then a 1x cache-reduce over 500 columns.
  - The target logit x_t is gathered on VectorE in one pass per slot:
    scalar_tensor_tensor  (iota64 == target_rel_p) * x  with accum_out,
    scanned only over the slot's 64-column window.
  - Tail per row:  pt = exp(x_t) * recip(sumexp)  (ScalarE Exp and
    VectorE reciprocal run in parallel), logpt = x_t - ln(sumexp); the
    gamma lookup telescopes  sum_b dg_b * [pt >= b/15]  with broadcast
    APs; loss = -(1 + eps - s*pt)^m * logpt via exp(m*ln(u)).  The tail
    runs in four unequal parts so only a tiny 4-slot part is exposed
    past the stream; its slots use ScalarE accum so the last sumexp
    lands early.  Each part's [128,1] row-sum partial is DMAed out as a
    column of the [128,4] result; the host sums and negates.

The gather windows are data-independent quantile bands (+-32 columns
~ 5.8 sigma of the sampling deviation for iid targets; the reference
distribution measures a max deviation of 26).  If an unusual
target distribution ever falls outside them, the host check catches it
and the kernel transparently rebuilds with full-width windows (slower
but always correct).
"""

import sys

for _p in ("/opt/trn_rl_repo",):
    if _p not in sys.path:
        sys.path.insert(0, _p)

import numpy as np

NUM_BINS = 15
EPS = 1e-20
N, C = 65536, 1000
NCORES = 8
NSHARD = N // NCORES  # 8192 rows per core
P = 128  # SBUF partitions
R = NSHARD // P  # 64 row-slots per partition
W = 64  # gather window width (columns) per row-slot
TAIL_BOUNDS = [0, 28, 48, 59, 64]  # unequal tail parts; only the last is exposed
NPART = len(TAIL_BOUNDS) - 1
ACT_ACC = 16  # total row-sums on ScalarE accum
# last-stretch engine pattern: neither engine alone sustains the arrival
# rate at the very end (A-slot costs ScalarE 1.39us, V-slot costs VectorE
# 1.39us, arrivals are 1.27us apart), so the final slots alternate; the
# very last is an A-slot so its sumexp lands 0.3us after its EXP
FORCED_ACT = {59, 61, 63}
TTR = False  # fused tensor_tensor_reduce rowsum (broken in this walrus: ISA length)
IO_BUFS = 10
EO_BUFS = 5
EARLY_ACT_RING = 2  # x DMAs issued on the Scalar HWDGE ring at t=0 so
# both HWDGE rings fill the SDMA queues from the first microsecond (0 =
# everything on the Sync ring; measured statistically equivalent)
CHUNK = 1  # slots per x DMA (1 = 512 KB transfers, 2 = 1 MB, ...)
SPLIT_FIRST = 2  # leading slots DMAed as 4 quarter-transfers each so the
# SDMA queues saturate from the first issue instead of ~1.5us later


def _slot_lo(w):
    # static window starts: slot s is centered on the s-th target quantile
    return [min(max(int(C * (s + 0.5) / R) - w // 2, 0), C - w) for s in range(R)]


def _split_excess_waits(nc, mybir, max_waits=1):
    """This container's walrus supports only one sync-wait command per
    instruction; hoist extra waits onto preceding same-engine no-ops."""
    ctr = 0
    for f in nc.m.functions:
        for bb in f.blocks:
            new_insts = []
            changed = False
            for inst in bb.instructions:
                si = inst.sync_info
                if si is not None and si.on_wait and len(si.on_wait) > max_waits:
                    waits = list(si.on_wait)
                    excess, keep = waits[:-max_waits], waits[-max_waits:]
                    for i in range(0, len(excess), max_waits):
                        ctr += 1
                        new_insts.append(
                            mybir.InstNoOp(
                                name=f"I-waitsplit-{ctr}",
                                sync_info=mybir.SyncInfo(
                                    on_wait=list(excess[i : i + max_waits]),
                                    on_update=[],
                                ),
                                bass_nofuse=True,
                                engine=inst.engine,
                            )
                        )
                    si.on_wait = keep
                    changed = True
                new_insts.append(inst)
            if changed:
                bb.instructions[:] = new_insts


def _build(w):
    import concourse.bass as bass
    import concourse.tile as tile
    from concourse import mybir

    f32 = mybir.dt.float32
    f16 = mybir.dt.float16
    AF = mybir.ActivationFunctionType
    ALU = mybir.AluOpType
    NB = NUM_BINS
    slot_lo = _slot_lo(w)
    CW = w + R + 3 * NB  # consts layout: iota64 | tmap_rel | ds | dm | thr

    nc = bass.Bass()
    x = nc.declare_dram_parameter("x", [NSHARD, C], f32, isOutput=False)
    consts = nc.declare_dram_parameter("consts", [P, CW], f32, isOutput=False)
    out = nc.declare_dram_parameter("out", [P, NPART], f32, isOutput=True)

    # target-sorted rank-major layout: HBM row s*128 + p holds the row for
    # slot s, partition p, so each CHUNK of slots is one contiguous DMA
    NCH = R // CHUNK
    x4 = x[:].rearrange("(u q p) c -> u p q c", u=NCH, q=CHUNK, p=P)

    # ScalarE-accum slots: the forced last-stretch pattern plus an even
    # spread of the rest over the earlier slots
    last_lo = TAIL_BOUNDS[NPART - 1]
    n_spread = max(ACT_ACC - len(FORCED_ACT), 0)
    act_slots = set(FORCED_ACT) | set(
        s
        for s in range(last_lo)
        if (s * n_spread) // last_lo != ((s + 1) * n_spread) // last_lo
    )

    def slot_part(slot):
        h = 0
        while slot >= TAIL_BOUNDS[h + 1]:
            h += 1
        return h, slot - TAIL_BOUNDS[h]

    part_w = [TAIL_BOUNDS[h + 1] - TAIL_BOUNDS[h] for h in range(NPART)]

    with tile.TileContext(nc) as tc:
        with (
            tc.tile_pool(name="const", bufs=1) as cpool,
            tc.tile_pool(name="io", bufs=IO_BUFS) as iopool,
            tc.tile_pool(name="escr", bufs=EO_BUFS) as epool,
            tc.tile_pool(name="sscr", bufs=3) as spool,
            tc.tile_pool(name="acc", bufs=1) as apool,
            tc.tile_pool(name="tail", bufs=2) as tpool,
        ):
            # EARLY_ACT_RING>0: first x chunks ride the Scalar HWDGE ring
            # so both SDMA queues fill from t=0 (but this delays ScalarE's
            # ACT-table load).  EARLY_ACT_RING=0: everything on the Sync
            # ring; two x chunks are still issued ahead of the consts.
            early = {}
            n_early = EARLY_ACT_RING if EARLY_ACT_RING > 0 else 2
            eng = nc.scalar if EARLY_ACT_RING > 0 else nc.sync
            for u in range(n_early):
                et = iopool.tile(
                    [P, CHUNK * C], f32, tag="xtile", name=f"xtile_e{u}"
                )
                eu = eng if (EARLY_ACT_RING == 0 or u % 2 == 0) else nc.sync
                if u < SPLIT_FIRST and CHUNK == 1:
                    q4 = C // 4
                    for qq in range(4):
                        eu.dma_start(
                            et[:, qq * q4 : (qq + 1) * q4],
                            x4[u, :, 0, qq * q4 : (qq + 1) * q4],
                        )
                else:
                    eu.dma_start(
                        et[:].rearrange("p (q c) -> p q c", q=CHUNK),
                        x4[u, :, :, :],
                    )
                early[u] = et

            ct = cpool.tile([P, CW], f32, tag="consts")
            eng.dma_start(ct[:], consts[:])
            iota64 = ct[:, 0:w]
            tmap_t = ct[:, w : w + R]
            ds = ct[:, w + R : w + R + NB]
            dm = ct[:, w + R + NB : w + R + 2 * NB]
            thr = ct[:, w + R + 2 * NB : w + R + 3 * NB]

            # per-part accumulators so each tail part only depends on its
            # own stretch of the main loop
            sumexp = [
                apool.tile([P, part_w[h]], f32, tag=f"sumexp{h}", name=f"sumexp{h}")
                for h in range(NPART)
            ]
            xt = [
                apool.tile([P, part_w[h]], f32, tag=f"xt{h}", name=f"xt{h}")
                for h in range(NPART)
            ]
            # all four per-part row-sum columns land here; ONE DMA at the
            # end ships them (a per-part DMA would sit in the Sync FIFO and
            # stall the x stream behind the part's tail chain)
            rs_all = apool.tile([P, NPART], f32, tag="rs_all", name="rs_all")

            def tail_part(h):
                se, xh = sumexp[h], xt[h]
                F = part_w[h]
                # pt = exp(x_t) * (1/sumexp): ScalarE and VectorE legs run
                # in parallel the moment this part's accumulators are done
                ext = tpool.tile([P, F], f32, tag="ext")
                nc.scalar.activation(ext[:], xh[:], AF.Exp)
                rse = tpool.tile([P, F], f32, tag="rse")
                nc.vector.reciprocal(rse[:], se[:])
                pt = tpool.tile([P, F], f32, tag="pt")
                nc.vector.tensor_mul(pt[:], ext[:], rse[:])
                lse = tpool.tile([P, F], f32, tag="lse")
                nc.scalar.activation(lse[:], se[:], AF.Ln)
                logpt = tpool.tile([P, F], f32, tag="logpt")
                nc.vector.tensor_sub(logpt[:], xh[:], lse[:])

                # s(pt), m(pt) via broadcast APs: ge[p,j,b] = pt[p,j]>=thr[p,b]
                ge = tpool.tile([P, F * NB], f32, tag="ge")
                ge3 = ge[:].rearrange("p (f b) -> p f b", b=NB)
                pt_b = (
                    pt[:]
                    .rearrange("p (f one) -> p f one", one=1)
                    .broadcast_to([P, F, NB])
                )
                thr_b = thr.rearrange("p (one b) -> p one b", one=1).broadcast_to(
                    [P, F, NB]
                )
                nc.vector.tensor_tensor(ge3, pt_b, thr_b, ALU.is_ge)
                ds_b = ds.rearrange("p (one b) -> p one b", one=1).broadcast_to(
                    [P, F, NB]
                )
                dm_b = dm.rearrange("p (one b) -> p one b", one=1).broadcast_to(
                    [P, F, NB]
                )
                prods = tpool.tile([P, F * NB], f32, tag="prods")
                nc.vector.tensor_tensor(
                    prods[:].rearrange("p (f b) -> p f b", b=NB), ge3, ds_b, ALU.mult
                )
                s_acc = tpool.tile([P, F], f32, tag="s_acc")
                nc.vector.tensor_reduce(
                    s_acc[:], prods[:].rearrange("p (f b) -> p f b", b=NB),
                    mybir.AxisListType.X, ALU.add,
                )
                prodm = tpool.tile([P, F * NB], f32, tag="prodm")
                nc.vector.tensor_tensor(
                    prodm[:].rearrange("p (f b) -> p f b", b=NB), ge3, dm_b, ALU.mult
                )
                m_acc = tpool.tile([P, F], f32, tag="m_acc")
                nc.vector.tensor_reduce(
                    m_acc[:], prodm[:].rearrange("p (f b) -> p f b", b=NB),
                    mybir.AxisListType.X, ALU.add,
                )

                # u = 1 + eps - s*pt ;  y = u^m = exp(m * ln(u)); the
                # "+1+eps" rides the Ln activation's bias immediate
                nspt = tpool.tile([P, F], f32, tag="nspt")
                nc.vector.scalar_tensor_tensor(
                    nspt[:], s_acc[:], -1.0, pt[:], ALU.mult, ALU.mult
                )
                v = tpool.tile([P, F], f32, tag="v")
                nc.scalar.activation(v[:], nspt[:], AF.Ln, bias=1.0 + EPS)
                w_ = tpool.tile([P, F], f32, tag="w")
                nc.vector.tensor_mul(w_[:], v[:], m_acc[:])
                y = tpool.tile([P, F], f32, tag="y")
                nc.scalar.activation(y[:], w_[:], AF.Exp)

                # per-partition partial of sum_j y*logpt (negated on host),
                # multiply and row-sum fused via STT accum_out
                prod = tpool.tile([P, F], f32, tag="prod")
                nc.vector.scalar_tensor_tensor(
                    prod[:], y[:], 1.0, logpt[:], ALU.mult, ALU.mult,
                    accum_out=rs_all[:, h : h + 1],
                )

            def do_slot(slot, xtile, off):
                h, col = slot_part(slot)
                eo = epool.tile([P, C], f16, tag="eo")
                if slot in act_slots:
                    nc.scalar.activation(
                        eo[:], xtile[:, off : off + C], AF.Exp,
                        accum_out=sumexp[h][:, col : col + 1],
                    )
                else:
                    nc.scalar.activation(eo[:], xtile[:, off : off + C], AF.Exp)
                    eh = epool.tile([P, C // 2], f16, tag="eh")
                    if TTR:
                        # halves-add + full-row reduce fused in one DVE op
                        nc.vector.tensor_tensor_reduce(
                            eh[:], eo[:, 0 : C // 2], eo[:, C // 2 : C],
                            1.0, 0.0, ALU.add, ALU.add,
                            accum_out=sumexp[h][:, col : col + 1],
                        )
                    else:
                        nc.vector.tensor_add(
                            eh[:], eo[:, 0 : C // 2], eo[:, C // 2 : C]
                        )
                        edum = epool.tile([P, C // 2], f16, tag="edum")
                        nc.vector.tensor_scalar(
                            edum[:], eh[:], 1.0, None,
                            ALU.mult, ALU.add,
                            accum_out=sumexp[h][:, col : col + 1],
                        )
                # rows are target-sorted, so this slot's targets all sit
                # inside a static window: the gather scans only it, against
                # window-relative indices (iota64 vs target - window_lo)
                lo = slot_lo[slot]
                so = spool.tile([P, w], f32, tag="so")
                nc.vector.scalar_tensor_tensor(
                    so[:],
                    iota64,
                    tmap_t[:, slot : slot + 1],
                    xtile[:, off + lo : off + lo + w],
                    ALU.is_equal,
                    ALU.mult,
                    accum_out=xt[h][:, col : col + 1],
                )

            done_parts = set()
            for u in range(NCH):
                if u in early:
                    xtile = early[u]
                else:
                    xtile = iopool.tile([P, CHUNK * C], f32, tag="xtile")
                    nc.sync.dma_start(
                        xtile[:].rearrange("p (q c) -> p q c", q=CHUNK),
                        x4[u, :, :, :],
                    )
                for q in range(CHUNK):
                    slot = u * CHUNK + q
                    do_slot(slot, xtile, q * C)
                    for hh in range(NPART - 1):
                        if slot >= TAIL_BOUNDS[hh + 1] - 1 and hh not in done_parts:
                            done_parts.add(hh)
                            tail_part(hh)  # overlaps the rest of the stream
            tail_part(NPART - 1)
            nc.sync.dma_start(out[:], rs_all[:])

    _split_excess_waits(nc, mybir, max_waits=1)
    return nc


_NC_CACHE = {}


def _get_nc(w):
    key = (
        w, CHUNK, ACT_ACC, TTR, EARLY_ACT_RING, SPLIT_FIRST,
        tuple(TAIL_BOUNDS), tuple(sorted(FORCED_ACT)),
    )
    if key not in _NC_CACHE:
        _NC_CACHE[key] = _build(w)
    return _NC_CACHE[key]


def _make_in_maps(input, target, gammas, w):
    inp = np.ascontiguousarray(np.asarray(input, dtype=np.float32))
    tgt = np.asarray(target).astype(np.int64)
    gam = np.asarray(gammas, dtype=np.float32)
    assert inp.shape == (N, C) and tgt.shape == (N,) and gam.shape == (NUM_BINS,)

    slot_lo = np.asarray(_slot_lo(w), dtype=np.int64)

    # host-precomputed constant block: iota64 | tmap_rel | ds | dm | thr
    sgn = np.sign(gam)
    mag = np.abs(gam)
    ds = np.concatenate([sgn[:1], sgn[1:] - sgn[:-1]]).astype(np.float32)
    dm = np.concatenate([mag[:1], mag[1:] - mag[:-1]]).astype(np.float32)
    thr = (np.arange(NUM_BINS, dtype=np.float32) / np.float32(NUM_BINS)).astype(
        np.float32
    )
    iota_w = np.arange(w, dtype=np.float32)

    in_maps = []
    for i in range(NCORES):
        tshard = tgt[NSHARD * i : NSHARD * (i + 1)]
        # sort rows by target; rank r -> slot r//P, partition r%P, so each
        # slot's 128 targets fall inside its static gather window
        order = np.argsort(tshard, kind="stable")
        tsorted = tshard[order]
        by_slot = tsorted.reshape(R, P)  # [slot, partition]
        lo = slot_lo[:, None]
        if not np.all((by_slot >= lo) & (by_slot <= lo + (w - 1))):
            return None  # caller falls back to full-width windows
        shard = np.ascontiguousarray(inp[NSHARD * i : NSHARD * (i + 1)][order])
        tmap_rel = (by_slot - lo).T.astype(np.float32)  # [P, R], in [0, w)
        row = np.concatenate([iota_w, np.zeros(R, np.float32), ds, dm, thr])
        consts = np.broadcast_to(row, (P, row.size)).copy()
        consts[:, w : w + R] = tmap_rel
        in_maps.append({"x": shard, "consts": np.ascontiguousarray(consts)})
    return in_maps


def kernel(input, target, gammas, _trace=False, _tmpdir=None):
    from concourse.bass_utils import run_bass_kernel_spmd

    in_maps = _make_in_maps(input, target, gammas, W)
    w = W
    if in_maps is None:
        # pathological target distribution: use full-width gather windows
        w = C
        in_maps = _make_in_maps(input, target, gammas, w)
        assert in_maps is not None  # w == C always satisfies the window check

    res = run_bass_kernel_spmd(
        _get_nc(w),
        in_maps,
        core_ids=list(range(NCORES)),
        trace=_trace,
        tmpdir=_tmpdir,
    )
    partials = [float(np.sum(res.results[i]["out"])) for i in range(NCORES)]
    total = -np.float32(np.sum(np.asarray(partials, dtype=np.float32)))
    if _trace:
        kernel._last_result = res
    return np.array(total, dtype=np.float32)


# revision 34
# speedup vs baseline: 1.0164x; 1.0164x over previous
"""AdaFocalLoss on 8 Trainium2 NeuronCores (Bass/Tile, SPMD).

Data-parallel over the batch axis, per the sharding hint: each core gets
8192 of the 65536 logit rows, the 15-entry gamma table is replicated, and
the per-core partial sums are combined on the host (the gather/unshard
step; the reduction over rows is order-independent).

Per-core kernel structure:
  - Rows are assigned to (slot, partition) SORTED BY TARGET on the host:
    slot s holds the 128 rows whose targets sit near the s-th quantile of
    the target distribution.  The row order is free to choose (the final
    loss is a sum over rows), and sorting makes the target-logit gather
    cheap: all 128 targets of a slot fall inside a static 64-column
    window around the slot's quantile center.
  - The shard streams as 64 contiguous 512 KB DMAs (one per slot).  The
    first two ride the Scalar engine's HWDGE ring while the rest queue on
    the Sync ring, so both rings fill the SDMA engines from t=0.  Every
    constant (window-relative iota, window-relative targets, telescoped
    gamma sign/magnitude deltas, bin thresholds) is precomputed on the
    host into one small [P, .] tensor - no device-side constant setup.
  - ScalarE computes exp(x) for every element (fp16 out; the only engine
    with transcendentals).  The per-row sum of exps comes from the
    ACTIVATE's accum_out for a few slots (cheap marginal cost) and from
    VectorE for the rest, as a two-stage reduce: a 2x-mode fp16 add of
    the tile's halves, then a 1x cache-reduce over 500 columns.
  - The target logit x_t is gathered on VectorE in one pass per slot:
    scalar_tensor_tensor  (iota64 == target_rel_p) * x  with accum_out,
    scanned only over the slot's 64-column window.
  - Tail per row:  pt = exp(x_t) * recip(sumexp)  (ScalarE Exp and
    VectorE reciprocal run in parallel), logpt = x_t - ln(sumexp); the
    gamma lookup telescopes  sum_b dg_b * [pt >= b/15]  with broadcast
    APs; loss = -(1 + eps - s*pt)^m * logpt via exp(m*ln(u)).  The tail
    runs in four unequal parts so only a tiny 4-slot part is exposed
    past the stream; its slots use ScalarE accum so the last sumexp
    lands early.  Each part's [128,1] row-sum partial is DMAed out as a
    column of the [128,4] result; the host sums and negates.

The gather windows are data-independent quantile bands (+-32 columns
~ 5.8 sigma of the sampling deviation for iid targets; the reference
distribution measures a max deviation of 26).  If an unusual
target distribution ever falls outside them, the host check catches it
and the kernel transparently rebuilds with full-width windows (slower
but always correct).
"""

import sys

for _p in ("/opt/trn_rl_repo",):
    if _p not in sys.path:
        sys.path.insert(0, _p)

import numpy as np

NUM_BINS = 15
EPS = 1e-20
N, C = 65536, 1000
NCORES = 8
NSHARD = N // NCORES  # 8192 rows per core
P = 128  # SBUF partitions
R = NSHARD // P  # 64 row-slots per partition
W = 64  # gather window width (columns) per row-slot
TAIL_BOUNDS = [0, 28, 48, 59, 64]  # unequal tail parts; only the last is exposed
NPART = len(TAIL_BOUNDS) - 1
ACT_ACC = 16  # total row-sums on ScalarE accum
# last-stretch engine pattern: neither engine alone sustains the arrival
# rate at the very end (A-slot costs ScalarE 1.39us, V-slot costs VectorE
# 1.39us, arrivals are 1.27us apart), so the final slots alternate; the
# very last is an A-slot so its sumexp lands 0.3us after its EXP
FORCED_ACT = {59, 61, 63}
TTR = False  # fused tensor_tensor_reduce rowsum (broken in this walrus: ISA length)
IO_BUFS = 10
EO_BUFS = 5
EARLY_ACT_RING = 2  # x DMAs issued on the Scalar HWDGE ring at t=0 so
# both HWDGE rings fill the SDMA queues from the first microsecond (0 =
# everything on the Sync ring; measured statistically equivalent)
CHUNK = 1  # slots per x DMA (1 = 512 KB transfers, 2 = 1 MB, ...)
SPLIT_FIRST = 0  # leading slots DMAed as 4 quarter-transfers each
# (measured ~2.3us WORSE: the strided quarter-rows cost more per packet
# than the empty-queue ramp they were meant to fill)


def _slot_lo(w):
    # static window starts: slot s is centered on the s-th target quantile
    return [min(max(int(C * (s + 0.5) / R) - w // 2, 0), C - w) for s in range(R)]


def _split_excess_waits(nc, mybir, max_waits=1):
    """This container's walrus supports only one sync-wait command per
    instruction; hoist extra waits onto preceding same-engine no-ops."""
    ctr = 0
    for f in nc.m.functions:
        for bb in f.blocks:
            new_insts = []
            changed = False
            for inst in bb.instructions:
                si = inst.sync_info
                if si is not None and si.on_wait and len(si.on_wait) > max_waits:
                    waits = list(si.on_wait)
                    excess, keep = waits[:-max_waits], waits[-max_waits:]
                    for i in range(0, len(excess), max_waits):
                        ctr += 1
                        new_insts.append(
                            mybir.InstNoOp(
                                name=f"I-waitsplit-{ctr}",
                                sync_info=mybir.SyncInfo(
                                    on_wait=list(excess[i : i + max_waits]),
                                    on_update=[],
                                ),
                                bass_nofuse=True,
                                engine=inst.engine,
                            )
                        )
                    si.on_wait = keep
                    changed = True
                new_insts.append(inst)
            if changed:
                bb.instructions[:] = new_insts


def _build(w):
    import concourse.bass as bass
    import concourse.tile as tile
    from concourse import mybir

    f32 = mybir.dt.float32
    f16 = mybir.dt.float16
    AF = mybir.ActivationFunctionType
    ALU = mybir.AluOpType
    NB = NUM_BINS
    slot_lo = _slot_lo(w)
    CW = w + R + 3 * NB  # consts layout: iota64 | tmap_rel | ds | dm | thr

    nc = bass.Bass()
    x = nc.declare_dram_parameter("x", [NSHARD, C], f32, isOutput=False)
    consts = nc.declare_dram_parameter("consts", [P, CW], f32, isOutput=False)
    out = nc.declare_dram_parameter("out", [P, NPART], f32, isOutput=True)

    # target-sorted rank-major layout: HBM row s*128 + p holds the row for
    # slot s, partition p, so each CHUNK of slots is one contiguous DMA
    NCH = R // CHUNK
    x4 = x[:].rearrange("(u q p) c -> u p q c", u=NCH, q=CHUNK, p=P)

    # ScalarE-accum slots: the forced last-stretch pattern plus an even
    # spread of the rest over the earlier slots
    last_lo = TAIL_BOUNDS[NPART - 1]
    n_spread = max(ACT_ACC - len(FORCED_ACT), 0)
    act_slots = set(FORCED_ACT) | set(
        s
        for s in range(last_lo)
        if (s * n_spread) // last_lo != ((s + 1) * n_spread) // last_lo
    )

    def slot_part(slot):
        h = 0
        while slot >= TAIL_BOUNDS[h + 1]:
            h += 1
        return h, slot - TAIL_BOUNDS[h]

    part_w = [TAIL_BOUNDS[h + 1] - TAIL_BOUNDS[h] for h in range(NPART)]

    with tile.TileContext(nc) as tc:
        with (
            tc.tile_pool(name="const", bufs=1) as cpool,
            tc.tile_pool(name="io", bufs=IO_BUFS) as iopool,
            tc.tile_pool(name="escr", bufs=EO_BUFS) as epool,
            tc.tile_pool(name="sscr", bufs=3) as spool,
            tc.tile_pool(name="acc", bufs=1) as apool,
            tc.tile_pool(name="tail", bufs=2) as tpool,
        ):
            # EARLY_ACT_RING>0: first x chunks ride the Scalar HWDGE ring
            # so both SDMA queues fill from t=0 (but this delays ScalarE's
            # ACT-table load).  EARLY_ACT_RING=0: everything on the Sync
            # ring; two x chunks are still issued ahead of the consts.
            early = {}
            n_early = EARLY_ACT_RING if EARLY_ACT_RING > 0 else 2
            eng = nc.scalar if EARLY_ACT_RING > 0 else nc.sync
            for u in range(n_early):
                et = iopool.tile(
                    [P, CHUNK * C], f32, tag="xtile", name=f"xtile_e{u}"
                )
                eu = eng
                if u < SPLIT_FIRST and CHUNK == 1:
                    q4 = C // 4
                    for qq in range(4):
                        eu.dma_start(
                            et[:, qq * q4 : (qq + 1) * q4],
                            x4[u, :, 0, qq * q4 : (qq + 1) * q4],
                        )
                else:
                    eu.dma_start(
                        et[:].rearrange("p (q c) -> p q c", q=CHUNK),
                        x4[u, :, :, :],
                    )
                early[u] = et

            ct = cpool.tile([P, CW], f32, tag="consts")
            eng.dma_start(ct[:], consts[:])
            iota64 = ct[:, 0:w]
            tmap_t = ct[:, w : w + R]
            ds = ct[:, w + R : w + R + NB]
            dm = ct[:, w + R + NB : w + R + 2 * NB]
            thr = ct[:, w + R + 2 * NB : w + R + 3 * NB]

            # per-part accumulators so each tail part only depends on its
            # own stretch of the main loop
            sumexp = [
                apool.tile([P, part_w[h]], f32, tag=f"sumexp{h}", name=f"sumexp{h}")
                for h in range(NPART)
            ]
            xt = [
                apool.tile([P, part_w[h]], f32, tag=f"xt{h}", name=f"xt{h}")
                for h in range(NPART)
            ]
            # all four per-part row-sum columns land here; ONE DMA at the
            # end ships them (a per-part DMA would sit in the Sync FIFO and
            # stall the x stream behind the part's tail chain)
            rs_all = apool.tile([P, NPART], f32, tag="rs_all", name="rs_all")

            def tail_part(h):
                se, xh = sumexp[h], xt[h]
                F = part_w[h]
                # pt = exp(x_t) * (1/sumexp): ScalarE and VectorE legs run
                # in parallel the moment this part's accumulators are done
                ext = tpool.tile([P, F], f32, tag="ext")
                nc.scalar.activation(ext[:], xh[:], AF.Exp)
                rse = tpool.tile([P, F], f32, tag="rse")
                nc.vector.reciprocal(rse[:], se[:])
                pt = tpool.tile([P, F], f32, tag="pt")
                nc.vector.tensor_mul(pt[:], ext[:], rse[:])
                lse = tpool.tile([P, F], f32, tag="lse")
                nc.scalar.activation(lse[:], se[:], AF.Ln)
                logpt = tpool.tile([P, F], f32, tag="logpt")
                nc.vector.tensor_sub(logpt[:], xh[:], lse[:])

                # s(pt), m(pt) via broadcast APs: ge[p,j,b] = pt[p,j]>=thr[p,b]
                ge = tpool.tile([P, F * NB], f32, tag="ge")
                ge3 = ge[:].rearrange("p (f b) -> p f b", b=NB)
                pt_b = (
                    pt[:]
                    .rearrange("p (f one) -> p f one", one=1)
                    .broadcast_to([P, F, NB])
                )
                thr_b = thr.rearrange("p (one b) -> p one b", one=1).broadcast_to(
                    [P, F, NB]
                )
                nc.vector.tensor_tensor(ge3, pt_b, thr_b, ALU.is_ge)
                ds_b = ds.rearrange("p (one b) -> p one b", one=1).broadcast_to(
                    [P, F, NB]
                )
                dm_b = dm.rearrange("p (one b) -> p one b", one=1).broadcast_to(
                    [P, F, NB]
                )
                prods = tpool.tile([P, F * NB], f32, tag="prods")
                nc.vector.tensor_tensor(
                    prods[:].rearrange("p (f b) -> p f b", b=NB), ge3, ds_b, ALU.mult
                )
                s_acc = tpool.tile([P, F], f32, tag="s_acc")
                nc.vector.tensor_reduce(
                    s_acc[:], prods[:].rearrange("p (f b) -> p f b", b=NB),
                    mybir.AxisListType.X, ALU.add,
                )
                prodm = tpool.tile([P, F * NB], f32, tag="prodm")
                nc.vector.tensor_tensor(
                    prodm[:].rearrange("p (f b) -> p f b", b=NB), ge3, dm_b, ALU.mult
                )
                m_acc = tpool.tile([P, F], f32, tag="m_acc")
                nc.vector.tensor_reduce(
                    m_acc[:], prodm[:].rearrange("p (f b) -> p f b", b=NB),
                    mybir.AxisListType.X, ALU.add,
                )

                # u = 1 + eps - s*pt ;  y = u^m = exp(m * ln(u)); the
                # "+1+eps" rides the Ln activation's bias immediate
                nspt = tpool.tile([P, F], f32, tag="nspt")
                nc.vector.scalar_tensor_tensor(
                    nspt[:], s_acc[:], -1.0, pt[:], ALU.mult, ALU.mult
                )
                v = tpool.tile([P, F], f32, tag="v")
                nc.scalar.activation(v[:], nspt[:], AF.Ln, bias=1.0 + EPS)
                w_ = tpool.tile([P, F], f32, tag="w")
                nc.vector.tensor_mul(w_[:], v[:], m_acc[:])
                y = tpool.tile([P, F], f32, tag="y")
                nc.scalar.activation(y[:], w_[:], AF.Exp)

                # per-partition partial of sum_j y*logpt (negated on host),
                # multiply and row-sum fused via STT accum_out
                prod = tpool.tile([P, F], f32, tag="prod")
                nc.vector.scalar_tensor_tensor(
                    prod[:], y[:], 1.0, logpt[:], ALU.mult, ALU.mult,
                    accum_out=rs_all[:, h : h + 1],
                )

            def do_slot(slot, xtile, off):
                h, col = slot_part(slot)
                eo = epool.tile([P, C], f16, tag="eo")
                if slot in act_slots:
                    nc.scalar.activation(
                        eo[:], xtile[:, off : off + C], AF.Exp,
                        accum_out=sumexp[h][:, col : col + 1],
                    )
                else:
                    nc.scalar.activation(eo[:], xtile[:, off : off + C], AF.Exp)
                    eh = epool.tile([P, C // 2], f16, tag="eh")
                    if TTR:
                        # halves-add + full-row reduce fused in one DVE op
                        nc.vector.tensor_tensor_reduce(
                            eh[:], eo[:, 0 : C // 2], eo[:, C // 2 : C],
                            1.0, 0.0, ALU.add, ALU.add,
                            accum_out=sumexp[h][:, col : col + 1],
                        )
                    else:
                        nc.vector.tensor_add(
                            eh[:], eo[:, 0 : C // 2], eo[:, C // 2 : C]
                        )
                        edum = epool.tile([P, C // 2], f16, tag="edum")
                        nc.vector.tensor_scalar(
                            edum[:], eh[:], 1.0, None,
                            ALU.mult, ALU.add,
                            accum_out=sumexp[h][:, col : col + 1],
                        )
                # rows are target-sorted, so this slot's targets all sit
                # inside a static window: the gather scans only it, against
                # window-relative indices (iota64 vs target - window_lo)
                lo = slot_lo[slot]
                so = spool.tile([P, w], f32, tag="so")
                nc.vector.scalar_tensor_tensor(
                    so[:],
                    iota64,
                    tmap_t[:, slot : slot + 1],
                    xtile[:, off + lo : off + lo + w],
                    ALU.is_equal,
                    ALU.mult,
                    accum_out=xt[h][:, col : col + 1],
                )

            done_parts = set()
            for u in range(NCH):
                if u in early:
                    xtile = early[u]
                else:
                    xtile = iopool.tile([P, CHUNK * C], f32, tag="xtile")
                    nc.sync.dma_start(
                        xtile[:].rearrange("p (q c) -> p q c", q=CHUNK),
                        x4[u, :, :, :],
                    )
                for q in range(CHUNK):
                    slot = u * CHUNK + q
                    do_slot(slot, xtile, q * C)
                    for hh in range(NPART - 1):
                        if slot >= TAIL_BOUNDS[hh + 1] - 1 and hh not in done_parts:
                            done_parts.add(hh)
                            tail_part(hh)  # overlaps the rest of the stream
            tail_part(NPART - 1)
            nc.sync.dma_start(out[:], rs_all[:])

    _split_excess_waits(nc, mybir, max_waits=1)
    return nc


_NC_CACHE = {}


def _get_nc(w):
    key = (
        w, CHUNK, ACT_ACC, TTR, EARLY_ACT_RING, SPLIT_FIRST,
        tuple(TAIL_BOUNDS), tuple(sorted(FORCED_ACT)),
    )
    if key not in _NC_CACHE:
        _NC_CACHE[key] = _build(w)
    return _NC_CACHE[key]


def _make_in_maps(input, target, gammas, w):
    inp = np.ascontiguousarray(np.asarray(input, dtype=np.float32))
    tgt = np.asarray(target).astype(np.int64)
    gam = np.asarray(gammas, dtype=np.float32)
    assert inp.shape == (N, C) and tgt.shape == (N,) and gam.shape == (NUM_BINS,)

    slot_lo = np.asarray(_slot_lo(w), dtype=np.int64)

    # host-precomputed constant block: iota64 | tmap_rel | ds | dm | thr
    sgn = np.sign(gam)
    mag = np.abs(gam)
    ds = np.concatenate([sgn[:1], sgn[1:] - sgn[:-1]]).astype(np.float32)
    dm = np.concatenate([mag[:1], mag[1:] - mag[:-1]]).astype(np.float32)
    thr = (np.arange(NUM_BINS, dtype=np.float32) / np.float32(NUM_BINS)).astype(
        np.float32
    )
    iota_w = np.arange(w, dtype=np.float32)

    in_maps = []
    for i in range(NCORES):
        tshard = tgt[NSHARD * i : NSHARD * (i + 1)]
        # sort rows by target; rank r -> slot r//P, partition r%P, so each
        # slot's 128 targets fall inside its static gather window
        order = np.argsort(tshard, kind="stable")
        tsorted = tshard[order]
        by_slot = tsorted.reshape(R, P)  # [slot, partition]
        lo = slot_lo[:, None]
        if not np.all((by_slot >= lo) & (by_slot <= lo + (w - 1))):
            return None  # caller falls back to full-width windows
        shard = np.ascontiguousarray(inp[NSHARD * i : NSHARD * (i + 1)][order])
        tmap_rel = (by_slot - lo).T.astype(np.float32)  # [P, R], in [0, w)
        row = np.concatenate([iota_w, np.zeros(R, np.float32), ds, dm, thr])
        consts = np.broadcast_to(row, (P, row.size)).copy()
        consts[:, w : w + R] = tmap_rel
        in_maps.append({"x": shard, "consts": np.ascontiguousarray(consts)})
    return in_maps


def kernel(input, target, gammas, _trace=False, _tmpdir=None):
    from concourse.bass_utils import run_bass_kernel_spmd

    in_maps = _make_in_maps(input, target, gammas, W)
    w = W
    if in_maps is None:
        # pathological target distribution: use full-width gather windows
        w = C
        in_maps = _make_in_maps(input, target, gammas, w)
        assert in_maps is not None  # w == C always satisfies the window check

    res = run_bass_kernel_spmd(
        _get_nc(w),
        in_maps,
        core_ids=list(range(NCORES)),
        trace=_trace,
        tmpdir=_tmpdir,
    )
    partials = [float(np.sum(res.results[i]["out"])) for i in range(NCORES)]
    total = -np.float32(np.sum(np.asarray(partials, dtype=np.float32)))
    if _trace:
        kernel._last_result = res
    return np.array(total, dtype=np.float32)
